# revision 19
# baseline (speedup 1.0000x reference)
"""MoLE linear (base W + shared LoRA + per-expert routed LoRA) on 8 TRN2 cores.

Math: out[b] = x[b] @ (W^T) + (x[b] @ A_cat) @ B_cat_scaled[b] + bias
  where A_cat = [A_s | A_e[0] | ... | A_e[7]]            -> [IN, 72]
        B_cat = [B_s ; B_e[0] ; ... ; B_e[7] ; bias]     -> [73, OUT]
        B_cat_scaled[b] rows 8+8e+r scaled by routing[b, e] (on device),
        and t^T gets an extra ones-row so row 72 of B_cat adds the bias.

Sharding: data-parallel over (batch, seq-half) -> 8 shards of [1024, IN]
tokens; W^T and the LoRA factors are replicated. Per core one fused kernel:
  t^T = A_cat^T @ x^T                    (K=i accumulation, PE)
  out[mtile] = sum_k x^T[k,m].T @ W^T[k,n]  (+ t^T[m].T @ B_cat[n] as the
              17th accumulating matmul into the same PSUM bank, which also
              applies routing weights and bias).
All matmuls run as float32r (full PE rate for free-dim >= 256).
Host does layout-only prep: transposes/concats; all arithmetic on device.
"""

import sys

import numpy as np

if "/opt/trn_rl_repo" not in sys.path:
    sys.path.insert(0, "/opt/trn_rl_repo")

IN = 2048
OUT = 2048
E = 8
R = 8
B = 4
S = 2048
NCORES = 8
S_SHARD = (B * S) // NCORES  # 1024 tokens per core
P = 128
NF = 512  # matmul moving free dim (one PSUM bank of fp32)
KT = IN // P  # 16 contraction tiles
MT = S_SHARD // P  # 8 output row tiles
NT = OUT // NF  # 4 output col tiles
RCAT = E * R + R  # 72 low-rank columns
RR = RCAT + 1  # + bias row

_NC = None


def _build_nc():
    import concourse.mybir as mybir
    from concourse import bacc
    from concourse.tile import TileContext

    f32 = mybir.dt.float32
    f32r = mybir.dt.float32r

    nc = bacc.Bacc(None, target_bir_lowering=False, debug=False)
    xT = nc.declare_dram_parameter("xT", [IN, S_SHARD], f32r, isOutput=False)
    wT = nc.declare_dram_parameter("wT", [IN, OUT], f32r, isOutput=False)
    # acat arrives pre-tiled as [P, KT*RCAT] (host layout) so the DMA has
    # one contiguous 4.6KB chunk per partition instead of 288B slivers.
    acat = nc.declare_dram_parameter("acat", [P, KT * RCAT], f32r, isOutput=False)
    bcat = nc.declare_dram_parameter("bcat", [RR, OUT], f32r, isOutput=False)
    rexp = nc.declare_dram_parameter("rexp", [RR, 1], f32, isOutput=False)
    ones = nc.declare_dram_parameter("ones", [1, S_SHARD], f32r, isOutput=False)
    out = nc.declare_dram_parameter("out", [S_SHARD, OUT], f32, isOutput=True)

    # DMA ring split (two HWDGE rings, FIFO per issuing engine):
    #   ACT ring (nc.scalar): acat, xT stream, ones, out stores  (~54 us)
    #   SP ring  (nc.sync):   rexp, bcat, W tile stream          (~53 us)
    with TileContext(nc) as tc:
        with (
            tc.tile_pool(name="const", bufs=1) as cpool,
            tc.tile_pool(name="w0p", bufs=1) as w0pool,
            tc.tile_pool(name="wtp", bufs=6) as wpool,
            tc.tile_pool(name="outp", bufs=4) as opool,
            tc.tile_pool(name="ps", bufs=8, space="PSUM") as pspool,
        ):
            M1 = 6  # row tiles finished during the phase-1 xT stream

            # ---- loads ordered for the phase-1 critical chain -------------
            # ACT ring: acat, xT[0], ones, xT[1..]; SP: wt0[0..1], rexp,
            # bcat, wt0[2..]. First t-matmul needs acat+xT[0] (~3.5us).
            acat_sb = cpool.tile([P, KT, RCAT], f32r)
            nc.scalar.dma_start(
                out=acat_sb[:], in_=acat[:].rearrange("p (kt r) -> p kt r", kt=KT)
            )
            acat_v = acat_sb

            xT_sb = cpool.tile([P, KT, S_SHARD], f32r)
            wt0 = [w0pool.tile([P, NF], f32r, name=f"wt0_{k}") for k in range(KT)]
            nc.scalar.dma_start(out=xT_sb[:, 0, :], in_=xT[0:P, :])
            nc.sync.dma_start(out=wt0[0][:], in_=wT[0:P, 0:NF])
            nc.sync.dma_start(out=wt0[1][:], in_=wT[P : 2 * P, 0:NF])

            tT_sb = cpool.tile([RR, S_SHARD], f32r)
            nc.scalar.dma_start(out=tT_sb[RCAT:RR, :], in_=ones[:])
            rexp_sb = cpool.tile([RR, 1], f32)
            nc.sync.dma_start(out=rexp_sb[:], in_=rexp[:])
            bcat_sb = cpool.tile([RR, OUT], f32r)
            nc.sync.dma_start(out=bcat_sb[:], in_=bcat[:])

            # routing weights (+ones for shared rows / bias row) scale B_cat
            bsc_sb = cpool.tile([RR, OUT], f32r)
            nc.vector.tensor_scalar_mul(bsc_sb[:], bcat_sb[:], rexp_sb[:])
            ps_t = [
                pspool.tile([RCAT, NF], f32, tag="ps", name=f"ps_t{n2}")
                for n2 in range(2)
            ]
            ps_o0 = [
                pspool.tile([P, NF], f32, tag="ps", name=f"ps_o0_{m}")
                for m in range(M1)
            ]

            def lora_mm(n, m, ps):
                # 17th accumulation: LoRA paths + routing + bias
                nc.tensor.matmul(
                    ps[:],
                    tT_sb[:, m * P : (m + 1) * P],
                    bsc_sb[:, n * NF : (n + 1) * NF],
                    start=False,
                    stop=True,
                )

            def store(n, m, ps):
                o_sb = opool.tile([P, NF], f32, tag="o", name=f"o{n}_{m}")
                nc.vector.tensor_copy(o_sb[:], ps[:])
                # alternate rings per row so each block's eviction drains on
                # two DMA chains instead of one
                eng = nc.scalar if (n + m) % 2 == 0 else nc.sync
                eng.dma_start(
                    out=out[m * P : (m + 1) * P, n * NF : (n + 1) * NF],
                    in_=o_sb[:],
                )

            # ---- phase 1: stream xT; per k do t-build + n=0 rows 0..5 -----
            # t^T = A_cat^T @ x^T -> [72, S_SHARD] (+ DMA'd ones row 72 so
            # the B_cat bias row adds the bias).
            for k in range(KT):
                if k > 0:
                    nc.scalar.dma_start(
                        out=xT_sb[:, k, :], in_=xT[k * P : (k + 1) * P, :]
                    )
                if k > 1:
                    nc.sync.dma_start(
                        out=wt0[k][:], in_=wT[k * P : (k + 1) * P, 0:NF]
                    )
                # base rows first: they only need xT[k]+wt0[k] (ready ~1.7us)
                # while the k=0 t-matmuls also wait on acat (~3.2us)
                for m in range(M1):
                    nc.tensor.matmul(
                        ps_o0[m][:],
                        xT_sb[:, k, m * P : (m + 1) * P],
                        wt0[k][:],
                        start=(k == 0),
                        stop=False,
                    )
                for n2 in range(2):
                    nc.tensor.matmul(
                        ps_t[n2][:],
                        acat_v[:, k, :],
                        xT_sb[:, k, n2 * NF : (n2 + 1) * NF],
                        start=(k == 0),
                        stop=(k == KT - 1),
                    )
            for n2 in range(2):
                nc.vector.tensor_copy(
                    tT_sb[0:RCAT, n2 * NF : (n2 + 1) * NF], ps_t[n2][:]
                )
            for m in range(M1):
                lora_mm(0, m, ps_o0[m])
                store(0, m, ps_o0[m])

            # ---- phase 1b: n=0 catch-up rows 6..7 (xT now resident) -------
            ps_o0b = [
                pspool.tile([P, NF], f32, tag="ps", name=f"ps_o0b_{m}")
                for m in range(M1, MT)
            ]
            for k in range(KT):
                for i, m in enumerate(range(M1, MT)):
                    nc.tensor.matmul(
                        ps_o0b[i][:],
                        xT_sb[:, k, m * P : (m + 1) * P],
                        wt0[k][:],
                        start=(k == 0),
                        stop=False,
                    )
                    if k == KT - 1:
                        lora_mm(0, m, ps_o0b[i])
                        store(0, m, ps_o0b[i])

            # ---- phases 2..4: n=1..3, W streamed on the SP ring -----------
            for n in range(1, NT):
                ps_o = [
                    pspool.tile([P, NF], f32, tag="ps", name=f"ps_o{n}_{m}")
                    for m in range(MT)
                ]
                for k in range(KT):
                    wt_sb = wpool.tile([P, NF], f32r, tag="wt", name=f"wt{n}_{k}")
                    nc.sync.dma_start(
                        out=wt_sb[:],
                        in_=wT[k * P : (k + 1) * P, n * NF : (n + 1) * NF],
                    )
                    for m in range(MT):
                        nc.tensor.matmul(
                            ps_o[m][:],
                            xT_sb[:, k, m * P : (m + 1) * P],
                            wt_sb[:],
                            start=(k == 0),
                            stop=False,
                        )
                        # on the last k, finish each row immediately so the
                        # eviction chain (DVE copy + out DMA) starts early
                        if k == KT - 1:
                            lora_mm(n, m, ps_o[m])
                            store(n, m, ps_o[m])
    nc.compile()
    return nc


def _get_nc():
    global _NC
    if _NC is None:
        _NC = _build_nc()
    return _NC


def _prep_in_maps(x, routing_weights, W, b, lora_A_s, lora_B_s, lora_A_e, lora_B_e):
    """Host-side layout-only prep (shard + transpose + concat)."""
    x = np.ascontiguousarray(np.asarray(x, dtype=np.float32))
    routing_weights = np.asarray(routing_weights, dtype=np.float32)
    W = np.asarray(W, dtype=np.float32)
    b = np.asarray(b, dtype=np.float32)
    lora_A_s = np.asarray(lora_A_s, dtype=np.float32)
    lora_B_s = np.asarray(lora_B_s, dtype=np.float32)
    lora_A_e = np.asarray(lora_A_e, dtype=np.float32)
    lora_B_e = np.asarray(lora_B_e, dtype=np.float32)

    wT = np.ascontiguousarray(W.T)  # [IN, OUT]
    acat = np.concatenate(
        [lora_A_s, lora_A_e.transpose(1, 0, 2).reshape(IN, E * R)], axis=1
    )  # [IN, 72]; col 8+8e+r = A_e[e,:,r]
    # pre-tile for the SBUF layout: [IN, 72] -> [P, KT*72]
    acat = np.ascontiguousarray(
        acat.reshape(KT, P, RCAT).transpose(1, 0, 2).reshape(P, KT * RCAT)
    )
    bcat = np.ascontiguousarray(
        np.concatenate([lora_B_s, lora_B_e.reshape(E * R, OUT), b[None, :]], axis=0)
    )  # [73, OUT]; row 8+8e+r = B_e[e,r,:], row 72 = bias

    # x -> [B, 2, S_SHARD, IN] -> per-shard transpose to [IN, S_SHARD]
    xs = x.reshape(B, NCORES // B, S_SHARD, IN)
    in_maps = []
    for c in range(NCORES):
        bi, h = divmod(c, NCORES // B)
        xT_shard = np.ascontiguousarray(xs[bi, h].T)  # [IN, S_SHARD]
        rexp = np.concatenate(
            [np.ones(R, np.float32), np.repeat(routing_weights[bi], R), np.ones(1, np.float32)]
        ).astype(np.float32)[:, None]  # [73, 1]
        in_maps.append(
            {
                "xT": xT_shard,
                "wT": wT,
                "acat": acat,
                "bcat": bcat,
                "rexp": rexp,
                "ones": np.ones((1, S_SHARD), dtype=np.float32),
            }
        )
    return in_maps


def _gather(results):
    out = np.empty((B, S, OUT), dtype=np.float32)
    for c in range(NCORES):
        bi, h = divmod(c, NCORES // B)
        out[bi, h * S_SHARD : (h + 1) * S_SHARD, :] = results[c]["out"]
    return out


def kernel(x, routing_weights, W, b, lora_A_s, lora_B_s, lora_A_e, lora_B_e):
    from concourse.bass_utils import run_bass_kernel_spmd

    nc = _get_nc()
    in_maps = _prep_in_maps(
        x, routing_weights, W, b, lora_A_s, lora_B_s, lora_A_e, lora_B_e
    )
    res = run_bass_kernel_spmd(nc, in_maps, list(range(NCORES))).results
    return _gather(res)


# revision 23
# speedup vs baseline: 4.4003x; 4.4003x over previous
"""MoLE linear (base W + shared LoRA + per-expert routed LoRA) on 8 TRN2 cores.

Math: out[b] = x[b] @ (W^T) + (x[b] @ A_cat) @ B_cat_scaled[b] + bias
  where A_cat = [A_s | A_e[0] | ... | A_e[7]]            -> [IN, 72]
        B_cat = [B_s ; B_e[0] ; ... ; B_e[7] ; bias]     -> [73, OUT]
        B_cat_scaled[b] rows 8+8e+r scaled by routing[b, e] (on device),
        and t^T gets an extra ones-row so row 72 of B_cat adds the bias.

Sharding: data-parallel over (batch, seq-half) -> 8 shards of [1024, IN]
tokens; W^T and the LoRA factors are replicated. Per core one fused kernel:
  t^T = A_cat^T @ x^T                    (K=i accumulation, PE)
  out[mtile] = sum_k x^T[k,m].T @ W^T[k,n]  (+ t^T[m].T @ B_cat[n] as the
              17th accumulating matmul into the same PSUM bank, which also
              applies routing weights and bias).
All matmuls run as float32r (full PE rate for free-dim >= 256).
Host does layout-only prep: transposes/concats; all arithmetic on device.
"""

import sys

import numpy as np

if "/opt/trn_rl_repo" not in sys.path:
    sys.path.insert(0, "/opt/trn_rl_repo")

IN = 2048
OUT = 2048
E = 8
R = 8
B = 4
S = 2048
NCORES = 8
S_SHARD = (B * S) // NCORES  # 1024 tokens per core
P = 128
NF = 512  # matmul moving free dim (one PSUM bank of fp32)
KT = IN // P  # 16 contraction tiles
MT = S_SHARD // P  # 8 output row tiles
NT = OUT // NF  # 4 output col tiles
RCAT = E * R + R  # 72 low-rank columns
RR = RCAT + 1  # + bias row

_NC = None


def _build_nc(chain=1):
    """Build the per-core program. chain>1 repeats the body (timing only)."""
    import concourse.mybir as mybir
    from concourse import bacc
    from concourse.tile import TileContext

    f32 = mybir.dt.float32
    f32r = mybir.dt.float32r

    nc = bacc.Bacc(None, target_bir_lowering=False, debug=False)
    xT = nc.declare_dram_parameter("xT", [IN, S_SHARD], f32r, isOutput=False)
    wT = nc.declare_dram_parameter("wT", [IN, OUT], f32r, isOutput=False)
    # acat arrives pre-tiled as [P, KT*RCAT] (host layout) so the DMA has
    # one contiguous 4.6KB chunk per partition instead of 288B slivers.
    acat = nc.declare_dram_parameter("acat", [P, KT * RCAT], f32r, isOutput=False)
    bcat = nc.declare_dram_parameter("bcat", [RR, OUT], f32r, isOutput=False)
    rexp = nc.declare_dram_parameter("rexp", [RR, 1], f32, isOutput=False)
    ones = nc.declare_dram_parameter("ones", [1, S_SHARD], f32r, isOutput=False)
    out = nc.declare_dram_parameter("out", [S_SHARD, OUT], f32, isOutput=True)

    # DMA ring split (two HWDGE rings, FIFO per issuing engine):
    #   ACT ring (nc.scalar): acat, xT stream, ones, out stores  (~54 us)
    #   SP ring  (nc.sync):   rexp, bcat, W tile stream          (~53 us)
    with TileContext(nc) as tc:
      for _rep in range(chain):
        with (
            tc.tile_pool(name="const", bufs=1) as cpool,
            tc.tile_pool(name="w0p", bufs=1) as w0pool,
            tc.tile_pool(name="wtp", bufs=6) as wpool,
            tc.tile_pool(name="outp", bufs=4) as opool,
            tc.tile_pool(name="ps", bufs=8, space="PSUM") as pspool,
        ):
            M1 = 6  # row tiles finished during the phase-1 xT stream

            # ---- loads ordered for the phase-1 critical chain -------------
            # ACT ring: acat, xT[0], ones, xT[1..]; SP: wt0[0..1], rexp,
            # bcat, wt0[2..]. First t-matmul needs acat+xT[0] (~3.5us).
            acat_sb = cpool.tile([P, KT, RCAT], f32r)
            nc.scalar.dma_start(
                out=acat_sb[:], in_=acat[:].rearrange("p (kt r) -> p kt r", kt=KT)
            )
            acat_v = acat_sb

            xT_sb = cpool.tile([P, KT, S_SHARD], f32r)
            wt0 = [w0pool.tile([P, NF], f32r, name=f"wt0_{k}") for k in range(KT)]
            nc.scalar.dma_start(out=xT_sb[:, 0, :], in_=xT[0:P, :])
            nc.sync.dma_start(out=wt0[0][:], in_=wT[0:P, 0:NF])
            nc.sync.dma_start(out=wt0[1][:], in_=wT[P : 2 * P, 0:NF])

            tT_sb = cpool.tile([RR, S_SHARD], f32r)
            nc.scalar.dma_start(out=tT_sb[RCAT:RR, :], in_=ones[:])
            rexp_sb = cpool.tile([RR, 1], f32)
            nc.sync.dma_start(out=rexp_sb[:], in_=rexp[:])
            bcat_sb = cpool.tile([RR, OUT], f32r)
            nc.sync.dma_start(out=bcat_sb[:], in_=bcat[:])

            # routing weights (+ones for shared rows / bias row) scale B_cat
            bsc_sb = cpool.tile([RR, OUT], f32r)
            nc.vector.tensor_scalar_mul(bsc_sb[:], bcat_sb[:], rexp_sb[:])
            ps_t = [
                pspool.tile([RCAT, NF], f32, tag="ps", name=f"ps_t{n2}")
                for n2 in range(2)
            ]
            ps_o0 = [
                pspool.tile([P, NF], f32, tag="ps", name=f"ps_o0_{m}")
                for m in range(M1)
            ]

            def lora_mm(n, m, ps):
                # 17th accumulation: LoRA paths + routing + bias
                nc.tensor.matmul(
                    ps[:],
                    tT_sb[:, m * P : (m + 1) * P],
                    bsc_sb[:, n * NF : (n + 1) * NF],
                    start=False,
                    stop=True,
                )

            def store(n, m, ps):
                o_sb = opool.tile([P, NF], f32, tag="o", name=f"o{n}_{m}")
                nc.vector.tensor_copy(o_sb[:], ps[:])
                # alternate rings per row so each block's eviction drains on
                # two DMA chains instead of one
                eng = nc.scalar if (n + m) % 2 == 0 else nc.sync
                eng.dma_start(
                    out=out[m * P : (m + 1) * P, n * NF : (n + 1) * NF],
                    in_=o_sb[:],
                )

            # ---- phase 1: stream xT; per k do t-build + n=0 rows 0..5 -----
            # t^T = A_cat^T @ x^T -> [72, S_SHARD] (+ DMA'd ones row 72 so
            # the B_cat bias row adds the bias).
            for k in range(KT):
                if k > 0:
                    nc.scalar.dma_start(
                        out=xT_sb[:, k, :], in_=xT[k * P : (k + 1) * P, :]
                    )
                if k > 1:
                    nc.sync.dma_start(
                        out=wt0[k][:], in_=wT[k * P : (k + 1) * P, 0:NF]
                    )
                # base rows first: they only need xT[k]+wt0[k] (ready ~1.7us)
                # while the k=0 t-matmuls also wait on acat (~3.2us)
                for m in range(M1):
                    nc.tensor.matmul(
                        ps_o0[m][:],
                        xT_sb[:, k, m * P : (m + 1) * P],
                        wt0[k][:],
                        start=(k == 0),
                        stop=False,
                    )
                for n2 in range(2):
                    nc.tensor.matmul(
                        ps_t[n2][:],
                        acat_v[:, k, :],
                        xT_sb[:, k, n2 * NF : (n2 + 1) * NF],
                        start=(k == 0),
                        stop=(k == KT - 1),
                    )
            for n2 in range(2):
                nc.vector.tensor_copy(
                    tT_sb[0:RCAT, n2 * NF : (n2 + 1) * NF], ps_t[n2][:]
                )
            for m in range(M1):
                lora_mm(0, m, ps_o0[m])
                store(0, m, ps_o0[m])

            # ---- phase 1b: n=0 catch-up rows 6..7 (xT now resident) -------
            ps_o0b = [
                pspool.tile([P, NF], f32, tag="ps", name=f"ps_o0b_{m}")
                for m in range(M1, MT)
            ]
            for k in range(KT):
                for i, m in enumerate(range(M1, MT)):
                    nc.tensor.matmul(
                        ps_o0b[i][:],
                        xT_sb[:, k, m * P : (m + 1) * P],
                        wt0[k][:],
                        start=(k == 0),
                        stop=False,
                    )
                    if k == KT - 1:
                        lora_mm(0, m, ps_o0b[i])
                        store(0, m, ps_o0b[i])

            # ---- phases 2..4: n=1..3, W streamed on the SP ring -----------
            for n in range(1, NT):
                ps_o = [
                    pspool.tile([P, NF], f32, tag="ps", name=f"ps_o{n}_{m}")
                    for m in range(MT)
                ]
                for k in range(KT):
                    wt_sb = wpool.tile([P, NF], f32r, tag="wt", name=f"wt{n}_{k}")
                    nc.sync.dma_start(
                        out=wt_sb[:],
                        in_=wT[k * P : (k + 1) * P, n * NF : (n + 1) * NF],
                    )
                    for m in range(MT):
                        nc.tensor.matmul(
                            ps_o[m][:],
                            xT_sb[:, k, m * P : (m + 1) * P],
                            wt_sb[:],
                            start=(k == 0),
                            stop=False,
                        )
                        # on the last k, finish each row immediately so the
                        # eviction chain (DVE copy + out DMA) starts early
                        if k == KT - 1:
                            lora_mm(n, m, ps_o[m])
                            store(n, m, ps_o[m])
    nc.compile()
    return nc


def _get_nc():
    global _NC
    if _NC is None:
        _NC = _build_nc()
    return _NC


def _prep_in_maps(x, routing_weights, W, b, lora_A_s, lora_B_s, lora_A_e, lora_B_e):
    """Host-side layout-only prep (shard + transpose + concat)."""
    x = np.ascontiguousarray(np.asarray(x, dtype=np.float32))
    routing_weights = np.asarray(routing_weights, dtype=np.float32)
    W = np.asarray(W, dtype=np.float32)
    b = np.asarray(b, dtype=np.float32)
    lora_A_s = np.asarray(lora_A_s, dtype=np.float32)
    lora_B_s = np.asarray(lora_B_s, dtype=np.float32)
    lora_A_e = np.asarray(lora_A_e, dtype=np.float32)
    lora_B_e = np.asarray(lora_B_e, dtype=np.float32)

    wT = np.ascontiguousarray(W.T)  # [IN, OUT]
    acat = np.concatenate(
        [lora_A_s, lora_A_e.transpose(1, 0, 2).reshape(IN, E * R)], axis=1
    )  # [IN, 72]; col 8+8e+r = A_e[e,:,r]
    # pre-tile for the SBUF layout: [IN, 72] -> [P, KT*72]
    acat = np.ascontiguousarray(
        acat.reshape(KT, P, RCAT).transpose(1, 0, 2).reshape(P, KT * RCAT)
    )
    bcat = np.ascontiguousarray(
        np.concatenate([lora_B_s, lora_B_e.reshape(E * R, OUT), b[None, :]], axis=0)
    )  # [73, OUT]; row 8+8e+r = B_e[e,r,:], row 72 = bias

    # x -> [B, 2, S_SHARD, IN] -> per-shard transpose to [IN, S_SHARD]
    xs = x.reshape(B, NCORES // B, S_SHARD, IN)
    in_maps = []
    for c in range(NCORES):
        bi, h = divmod(c, NCORES // B)
        xT_shard = np.ascontiguousarray(xs[bi, h].T)  # [IN, S_SHARD]
        rexp = np.concatenate(
            [np.ones(R, np.float32), np.repeat(routing_weights[bi], R), np.ones(1, np.float32)]
        ).astype(np.float32)[:, None]  # [73, 1]
        in_maps.append(
            {
                "xT": xT_shard,
                "wT": wT,
                "acat": acat,
                "bcat": bcat,
                "rexp": rexp,
                "ones": np.ones((1, S_SHARD), dtype=np.float32),
            }
        )
    return in_maps


def _gather(results):
    out = np.empty((B, S, OUT), dtype=np.float32)
    for c in range(NCORES):
        bi, h = divmod(c, NCORES // B)
        out[bi, h * S_SHARD : (h + 1) * S_SHARD, :] = results[c]["out"]
    return out


def kernel(x, routing_weights, W, b, lora_A_s, lora_B_s, lora_A_e, lora_B_e):
    from concourse.bass_utils import run_bass_kernel_spmd

    nc = _get_nc()
    in_maps = _prep_in_maps(
        x, routing_weights, W, b, lora_A_s, lora_B_s, lora_A_e, lora_B_e
    )
    res = run_bass_kernel_spmd(nc, in_maps, list(range(NCORES))).results
    return _gather(res)
